# revision 1
# baseline (speedup 1.0000x reference)
"""CTC loss (keras ctc_batch_cost semantics) on 8 Trainium2 NeuronCores.

Strategy (pure data parallelism, batch sharded 128 samples/core):
  - DP runs in probability space with periodic per-sample rescaling:
        P[t,s] = y_ext[t,s] * (P[t-1,s] + P[t-1,s-1] + allow_skip*P[t-1,s-2])
    Samples ride the 128 SBUF partitions; the S=129 lattice states live in
    the free dimension of [128, S]-shaped DVE ops.
  - The per-(sample,t) emission gather y_pred[b,t,ext(b,s)] is done with
    per-sample one-hot matmuls on the PE array:
        PE transpose  y[b]  [T,C] -> [C,T]   (128x128 blocks)
        G[b] = W[b].T @ yT[b]   with W[b] [C,128] = packed one-hots:
            cols 0..63  : onehot(lab[l])                (odd-state emissions)
            cols 64..127: onehot(lab[l]) * allow_skip   (skip-masked copy)
    Per time step a second PE transpose turns G[:, t-slice, b] into a
    [128b, 128m] tile the DVE consumes directly from PSUM.
  - Blank emissions (even lattice states) multiply by a per-partition scalar
    plane ybe[b,t] = y_pred[b,t,C-1]+EPS (ScalarE activation with scale-AP).
  - Loss = -(log(P[2L] + P[2L-1]) + sum of rescale logs).
"""

import numpy as np

B, T, C, L = 1024, 512, 256, 64
S = 2 * L + 1  # 129
NCORES = 8
BL = B // NCORES  # 128 samples per core
EPS = 1e-7
RBLK = 8  # rescale period (time steps)
# Static per-state exponential tilt P~[s] = P[s]*exp(-G_TILT*s). Flattens the
# lattice's s-profile so all answer-relevant states fit f32 range; folded into
# the sh1 scalar, the host-built W2/end-mask, and the logacc initialization.
G_TILT = 1.75
OFFS = 30.0  # rescale offset: row max is normalized to e^OFFS, not 1

_prog = None  # cached compiled Bass program
_last_results = None


def _build_program():
    from contextlib import ExitStack

    import concourse.bacc as bacc
    import concourse.bass as bass
    import concourse.mybir as mybir
    import concourse.tile as tile

    F32 = mybir.dt.float32
    BF16 = mybir.dt.bfloat16
    OP = mybir.AluOpType
    AF = mybir.ActivationFunctionType
    AX = mybir.AxisListType
    PSUM = bass.MemorySpace.PSUM

    TCH = 128            # time-chunk length
    NCH = T // TCH       # 4 chunks
    NQ = BL // 4         # sample quads per chunk
    E1 = float(np.exp(-G_TILT))
    OFFE = float(np.exp(OFFS))

    nc = bacc.Bacc("TRN2", target_bir_lowering=False, debug=False)

    yp_d = nc.dram_tensor("yp", [BL, T, C], BF16, kind="ExternalInput").ap()
    wg_d = nc.dram_tensor("wg", [BL // 4, 128, 4, 256], BF16, kind="ExternalInput").ap()
    ybe_d = nc.dram_tensor("ybe", [BL, T], F32, kind="ExternalInput").ap()
    em_d = nc.dram_tensor("em", [BL, S], F32, kind="ExternalInput").ap()
    idf_d = nc.dram_tensor("idf", [128, 128], BF16, kind="ExternalInput").ap()
    we_d = nc.dram_tensor("we", [1, BL * 128], BF16, kind="ExternalInput").ap()
    pend_d = nc.dram_tensor("pend", [BL, 1], F32, kind="ExternalOutput").ap()
    mxh_d = nc.dram_tensor("mxh", [BL, T // RBLK], F32, kind="ExternalOutput").ap()

    with tile.TileContext(nc) as tc, ExitStack() as ctx:
        # ---- persistent SBUF state (one pool, unique tags) ----
        per = ctx.enter_context(tc.tile_pool(name="per", bufs=1))
        ybe_sb = per.tile([128, T], F32, tag="ybe", name="ybe_sb")
        em_sb = per.tile([128, S], F32, tag="em", name="em_sb")
        idf = per.tile([128, 128], BF16, tag="idf", name="idf_sb")
        pa = per.tile([128, 264], F32, tag="pa", name="pa")
        pb = per.tile([128, 264], F32, tag="pb", name="pb")
        mxh = per.tile([128, T // RBLK], F32, tag="mxh", name="mxh")
        we_sb = per.tile([1, BL * 128], BF16, tag="we", name="we_sb")
        ones_sb = per.tile([1, 128], BF16, tag="ones", name="ones_sb")

        nc.sync.dma_start(we_sb[:], we_d)
        nc.vector.memset(ones_sb[:], 1.0)
        nc.sync.dma_start(ybe_sb[:], ybe_d)
        nc.sync.dma_start(em_sb[:], em_d)
        nc.sync.dma_start(idf[:], idf_d)
        nc.vector.memset(pa[:], 0.0)
        nc.vector.memset(pb[:], 0.0)

        # ---- pools ----
        ytp = ctx.enter_context(tc.tile_pool(name="ytp", bufs=16))
        wpl = ctx.enter_context(tc.tile_pool(name="wpl", bufs=6))
        gcp = ctx.enter_context(tc.tile_pool(name="gcp", bufs=3))
        apl = ctx.enter_context(tc.tile_pool(name="apl", bufs=3))
        vpl = ctx.enter_context(tc.tile_pool(name="vpl", bufs=3))
        spl = ctx.enter_context(tc.tile_pool(name="spl", bufs=6))
        gpp = ctx.enter_context(tc.tile_pool(name="gpp", space=PSUM, bufs=3))
        yyp = ctx.enter_context(tc.tile_pool(name="yyp", space=PSUM, bufs=4))

        gc3 = {}  # chunk -> [128m, TCH, 128b] SBUF view (bf16)

        def gather_open(k):
            g = gcp.tile([128, TCH * 128], BF16, tag="gc")
            g3 = g[:].rearrange("p (t b) -> p t b", b=128)
            gc3[k] = g3

        def gather_quad(k, q):
            g3 = gc3[k]
            if True:
                w = wpl.tile([128, 4 * 256], BF16, tag="w")
                nc.scalar.dma_start(w[:], wg_d[q].rearrange("c si m -> c (si m)"))
                w4 = w[:].rearrange("c (si m) -> c si m", si=4)
                yts = []
                for si in range(4):
                    smp = q * 4 + si
                    yt0 = ytp.tile([128, TCH], BF16, tag="yt")
                    yt1 = ytp.tile([128, TCH], BF16, tag="yt")
                    nc.sync.dma_start(yt0[:], yp_d[smp, k * TCH:(k + 1) * TCH, 0:128],
                                      transpose=True)
                    nc.sync.dma_start(yt1[:], yp_d[smp, k * TCH:(k + 1) * TCH, 128:256],
                                      transpose=True)
                    yts.append((yt0, yt1))
                gq = gpp.tile([128, 512], F32, tag="gq")
                for si in range(4):
                    smp = q * 4 + si
                    sl = slice(si * 128, (si + 1) * 128)
                    yt0, yt1 = yts[si]
                    nc.tensor.matmul(gq[:, sl], w4[:, si, 0:128], yt0[:], start=True, stop=False)
                    nc.tensor.matmul(gq[:, sl], w4[:, si, 128:256], yt1[:], start=False, stop=False)
                    # +EPS via a K=1 ones-row matmul (host-scaled column sums)
                    nc.tensor.matmul(gq[:, sl],
                                     we_sb[0:1, smp * 128:(smp + 1) * 128],
                                     ones_sb[:], start=False, stop=True)
                # one strided copy: [128m,(si,t)] -> G[128m, t, 4b] at b-offset 4q
                gq3 = gq[:].rearrange("p (si t) -> p si t", si=4)
                outv = g3[:, :, q * 4:q * 4 + 4].rearrange("p t b -> p b t")
                nc.scalar.activation(outv, gq3, AF.Copy, bias=0.0)

        def gather_chunk(k):
            gather_open(k)
            for q in range(NQ):
                gather_quad(k, q)

        AOFF = 134  # A[s] lives at col AOFF+s of the *current* state tensor

        def dp_step(t, pcur, pnxt, rec2):
            k, tl = divmod(t, TCH)
            yy = yyp.tile([128, 128], BF16, tag="yy")
            nc.tensor.transpose(yy[:], gc3[k][:, tl, :], idf[:])
            # A[s] = P[s] + e^-g*P[s-1], written into pcur's scratch region
            nc.vector.scalar_tensor_tensor(pcur[:, AOFF:AOFF + 129],
                                           pcur[:, 0:129], E1,
                                           pcur[:, 1:130], OP.mult, OP.add)
            u3 = pnxt[:, 1:131].rearrange("p (s two) -> p s two", two=2)
            a_even = pcur[:, AOFF:AOFF + 130].rearrange(
                "p (s two) -> p s two", two=2)[:, :, 0]
            # even states: (A_even * ybe) [* rec2 on post-rescale steps]
            if rec2 is None:
                nc.vector.tensor_scalar(u3[:, :, 0], a_even, ybe_sb[:, t:t + 1],
                                        None, OP.mult)
            else:
                nc.vector.tensor_scalar(u3[:, :, 0], a_even, ybe_sb[:, t:t + 1],
                                        rec2[:], OP.mult, OP.mult)
            # one 2D-strided multiply covers skip & label terms:
            #   X[0,l] = P[2l]     * yy[0..63]   (skip: e^-2g * masked onehot)
            #   X[1,l] = A[2l+1]   * yy[64..127] (label emission)
            stz = bass.AP(pcur[:].tensor, pcur[:].offset,
                          [pcur[:].ap[0], [AOFF + 1, 2], [2, 64]])
            x = vpl.tile([128, 128], F32, tag="x")
            if rec2 is None:
                nc.vector.tensor_tensor(x[:], stz, yy[:], OP.mult)
            else:
                nc.vector.scalar_tensor_tensor(x[:], stz, rec2[:], yy[:],
                                               OP.mult, OP.mult)
            nc.vector.tensor_tensor(u3[:, 0:64, 1], x[:, 0:64], x[:, 64:128],
                                    OP.add)
            if t % RBLK == RBLK - 1:
                ridx = t // RBLK
                mxc = mxh[:, ridx:ridx + 1]
                nc.vector.tensor_reduce(mxc, pnxt[:, 1:130], AX.X, OP.max)
                rec = spl.tile([128, 1], F32, tag="rec")
                nc.vector.reciprocal(rec[:], mxc)
                rec2n = spl.tile([128, 1], F32, tag="rec2")
                nc.vector.tensor_scalar(rec2n[:], rec[:], OFFE, None, OP.mult)
                return rec2n
            return None

        gather_chunk(0)

        # init (t = 0): P[s=0] = ybe[:,0]; P~[s=1] = e^-g * y_lab(l=0,t=0)
        yy0 = yyp.tile([128, 128], BF16, tag="yy")
        nc.tensor.transpose(yy0[:], gc3[0][:, 0, :], idf[:])
        nc.vector.tensor_copy(pa[:, 1:2], ybe_sb[:, 0:1])
        nc.vector.tensor_scalar(pa[:, 2:3], yy0[:, 64:65], E1, None, OP.mult)

        pcur, pnxt = pa, pb
        rec2 = None
        for t in range(1, T):
            k, tl = divmod(t, TCH)
            # interleave next-chunk gather emission through this chunk's DP
            # steps so every engine's program order alternates DP and gather
            if k + 1 < NCH:
                if tl == 1:
                    gather_open(k + 1)
                if tl % 4 == 1:
                    gather_quad(k + 1, tl // 4)
            rec2 = dp_step(t, pcur, pnxt, rec2)
            pcur, pnxt = pnxt, pcur
        if rec2 is not None:
            # the last rescale's scaling never got absorbed; apply it now
            nc.vector.tensor_scalar_mul(pcur[:, 1:130], pcur[:, 1:130], rec2[:])

        # final: export pend = sum(P * endmask) and the rescale history;
        # the exact logs happen on the host.
        scre = per.tile([128, S], F32, tag="scre", name="scre")
        nc.vector.tensor_tensor(scre[:], pcur[:, 1:130], em_sb[:], OP.mult)
        pend = per.tile([128, 1], F32, tag="pend", name="pend")
        nc.vector.tensor_reduce(pend[:], scre[:], AX.X, OP.add)
        nc.sync.dma_start(pend_d, pend[:])
        nc.sync.dma_start(mxh_d, mxh[:])

    nc.compile()
    return nc


def _host_derived(y_true, y_pred, label_length):
    import ml_dtypes

    lab = np.asarray(y_true, dtype=np.int64)  # [B, 64]
    llv = np.asarray(label_length).reshape(-1)
    # packed one-hots: [B, C, 128]; cols 0..63 labels (validity-masked),
    # cols 64..127 skip-masked labels scaled by e^(-2g)
    vm = (np.arange(L)[None, :] < llv[:, None])  # valid odd state s=2l+1
    zm = np.concatenate([np.zeros((B, 1), bool), lab[:, 1:] != lab[:, :-1]], axis=1)
    w = np.zeros((B, C, 128), dtype=np.float32)
    bb = np.repeat(np.arange(B), L)
    ll = np.tile(np.arange(L), B)
    cc = lab.reshape(-1)
    w[bb, cc, L + ll] = vm.reshape(-1).astype(np.float32)
    w[bb, cc, ll] = np.where(
        (zm & vm).reshape(-1),
        np.float32(np.exp(-2.0 * G_TILT)),
        w[bb, cc, ll],
    )
    # device layout: [quad, 128c(lo), 4si, (ck m)] with c = ck*128 + c_lo
    w5 = w.reshape(B // 4, 4, 2, 128, 128)          # [q, si, ck, c_lo, m]
    w5 = w5.transpose(0, 3, 1, 2, 4)                # [q, c_lo, si, ck, m]
    wg = np.ascontiguousarray(
        w5.reshape(B // 4, 128, 4, 256).astype(ml_dtypes.bfloat16)
    )
    we = np.ascontiguousarray(
        (np.float32(EPS) * w.sum(axis=1)).astype(ml_dtypes.bfloat16).reshape(1, -1)
    )
    ybe = np.ascontiguousarray(np.asarray(y_pred)[:, :, C - 1] + np.float32(EPS))
    return wg, we, ybe


def kernel(y_true, y_pred, input_length, label_length, _trace=False):
    global _prog, _last_results
    from concourse.bass_utils import run_bass_kernel_spmd

    y_true = np.asarray(y_true)
    import ml_dtypes
    y_pred = np.asarray(y_pred, dtype=np.float32)
    y_pred_bf = y_pred.astype(ml_dtypes.bfloat16)
    label_length = np.asarray(label_length).reshape(-1)

    wg, we, ybe = _host_derived(y_true, y_pred, label_length)
    em = np.zeros((B, S), dtype=np.float32)
    bidx = np.arange(B)
    em[bidx, 2 * label_length] = 1.0
    em[bidx, 2 * label_length - 1] = np.float32(np.exp(-G_TILT))
    import ml_dtypes as _mld
    idf = np.eye(128, dtype=_mld.bfloat16)

    if _prog is None:
        _prog = _build_program()

    in_maps = []
    for i in range(NCORES):
        sl = slice(i * BL, (i + 1) * BL)
        slq = slice(i * (BL // 4), (i + 1) * (BL // 4))
        in_maps.append({
            "yp": np.ascontiguousarray(y_pred_bf[sl]),
            "wg": wg[slq],
            "ybe": ybe[sl],
            "em": em[sl],
            "we": we[:, i * BL * 128:(i + 1) * BL * 128],
            "idf": idf,
        })
    res = run_bass_kernel_spmd(_prog, in_maps, core_ids=list(range(NCORES)),
                               trace=_trace)
    _last_results = res
    pend = np.concatenate([r["pend"] for r in res.results], axis=0).reshape(-1)
    mxh = np.concatenate([r["mxh"] for r in res.results], axis=0)
    nres = mxh.shape[1]
    logacc = np.log(mxh.astype(np.float64)).sum(axis=1) - OFFS * nres
    loss = -(np.log(pend.astype(np.float64)) + logacc
             + G_TILT * 2.0 * label_length.astype(np.float64))
    return loss.reshape(B, 1).astype(np.float32)


if __name__ == "__main__":
    rng = np.random.default_rng(0)
    yp = rng.random((B, T, C), dtype=np.float32)
    yp /= yp.sum(-1, keepdims=True)
    yt = rng.integers(0, C - 1, size=(B, L)).astype(np.int32)
    il = np.full((B, 1), T, dtype=np.int32)
    ll = rng.integers(32, L + 1, size=(B, 1)).astype(np.int32)
    print(kernel(yt, yp, il, ll)[:4])



# revision 6
# speedup vs baseline: 3.7320x; 3.7320x over previous
"""CTC loss (keras ctc_batch_cost semantics) on 8 Trainium2 NeuronCores.

Strategy (pure data parallelism, batch sharded 128 samples/core):
  - DP runs in probability space with periodic per-sample rescaling:
        P[t,s] = y_ext[t,s] * (P[t-1,s] + P[t-1,s-1] + allow_skip*P[t-1,s-2])
    Samples ride the 128 SBUF partitions; the S=129 lattice states live in
    the free dimension of [128, S]-shaped DVE ops.
  - The per-(sample,t) label emissions y_pred[b,t,lab(b,l)] are pre-gathered
    on the host into a per-step dictionary ylc[b, t, 0:128]:
        cols 0..63  : e^(-2g) * allow_skip * (y_lab + EPS)   (skip term)
        cols 64..127: valid * (y_lab + EPS)                  (label emission)
    so the device loop is pure DVE work on SBUF-resident tiles - no
    gather matmuls, no transposes, no PSUM.
  - Blank emissions (even lattice states) multiply by a per-partition scalar
    plane ybe[b,t] = y_pred[b,t,C-1]+EPS (tensor_scalar with scalar-AP).
  - Loss = -(log(P[2L] + P[2L-1]) + sum of rescale logs).
"""

import numpy as np

B, T, C, L = 1024, 512, 256, 64
S = 2 * L + 1  # 129
NCORES = 8
BL = B // NCORES  # 128 samples per core
EPS = 1e-7
RBLK = 8  # rescale period (time steps)
# Static per-state exponential tilt P~[s] = P[s]*exp(-G_TILT*s). Flattens the
# lattice's s-profile so all answer-relevant states fit f32 range; folded into
# the sh1 scalar, the host-built dictionary/end-mask, and the final log.
G_TILT = 1.75

_prog = None  # cached compiled Bass program
_last_results = None


def _build_program():
    from contextlib import ExitStack

    import concourse.bacc as bacc
    import concourse.bass as bass
    import concourse.mybir as mybir
    import concourse.tile as tile

    F32 = mybir.dt.float32
    BF16 = mybir.dt.bfloat16
    OP = mybir.AluOpType
    AF = mybir.ActivationFunctionType
    AX = mybir.AxisListType

    TCH = 128            # time-chunk length (per input DMA)
    NCH = T // TCH       # 4 chunks
    E1 = float(np.exp(-G_TILT))

    nc = bacc.Bacc("TRN2", target_bir_lowering=False, debug=False)

    ylc_d = nc.dram_tensor("ylc", [BL, T, 128], BF16, kind="ExternalInput").ap()
    ybe_d = nc.dram_tensor("ybe", [BL, T], F32, kind="ExternalInput").ap()
    em_d = nc.dram_tensor("em", [BL, S], F32, kind="ExternalInput").ap()
    pend_d = nc.dram_tensor("pend", [BL, 1], F32, kind="ExternalOutput").ap()
    mxh_d = nc.dram_tensor("mxh", [BL, T // RBLK], F32, kind="ExternalOutput").ap()

    with tile.TileContext(nc) as tc, ExitStack() as ctx:
        # ---- persistent SBUF state (one pool, unique tags) ----
        per = ctx.enter_context(tc.tile_pool(name="per", bufs=1))
        ybe_sb = per.tile([128, T], F32, tag="ybe", name="ybe_sb")
        em_sb = per.tile([128, S], F32, tag="em", name="em_sb")
        pa = per.tile([128, 264], F32, tag="pa", name="pa")
        pb = per.tile([128, 264], F32, tag="pb", name="pb")
        mxh = per.tile([128, T // RBLK], F32, tag="mxh", name="mxh")
        ylcs = [per.tile([128, TCH * 128], BF16, tag=f"ylc{k}", name=f"ylc{k}")
                for k in range(NCH)]

        nc.sync.dma_start(ybe_sb[:], ybe_d)
        nc.sync.dma_start(em_sb[:], em_d)
        nc.vector.memset(pa[:], 0.0)
        nc.vector.memset(pb[:], 0.0)
        for k in range(NCH):
            nc.sync.dma_start(
                ylcs[k][:],
                ylc_d[:, k * TCH:(k + 1) * TCH, :].rearrange("p t c -> p (t c)"))

        # ---- pools ----
        vpl = ctx.enter_context(tc.tile_pool(name="vpl", bufs=3))
        spl = ctx.enter_context(tc.tile_pool(name="spl", bufs=6))

        AOFF = 134  # A[s] lives at col AOFF+s of the *current* state tensor

        def dp_step(t, pcur, pnxt, rec2):
            k, tl = divmod(t, TCH)
            yy = ylcs[k][:, tl * 128:(tl + 1) * 128]
            # A[s] = P[s] + e^-g*P[s-1], written into pcur's scratch region
            nc.vector.scalar_tensor_tensor(pcur[:, AOFF:AOFF + 129],
                                           pcur[:, 0:129], E1,
                                           pcur[:, 1:130], OP.mult, OP.add)
            u3 = pnxt[:, 1:131].rearrange("p (s two) -> p s two", two=2)
            a_even = pcur[:, AOFF:AOFF + 130].rearrange(
                "p (s two) -> p s two", two=2)[:, :, 0]
            # even states: (A_even * ybe) [* rec2 on post-rescale steps]
            if rec2 is None:
                nc.vector.tensor_scalar(u3[:, :, 0], a_even, ybe_sb[:, t:t + 1],
                                        None, OP.mult)
            else:
                nc.vector.tensor_scalar(u3[:, :, 0], a_even, ybe_sb[:, t:t + 1],
                                        rec2[:], OP.mult, OP.mult)
            # one 2D-strided multiply covers skip & label terms:
            #   X[0,l] = P[2l-1] * yy[0..63]   (skip: e^-2g * masked emission)
            #   X[1,l] = A[2l+1] * yy[64..127] (label emission)
            stz = bass.AP(pcur[:].tensor, pcur[:].offset,
                          [pcur[:].ap[0], [AOFF + 1, 2], [2, 64]])
            x = vpl.tile([128, 128], F32, tag="x")
            if rec2 is None:
                nc.vector.tensor_tensor(x[:], stz, yy, OP.mult)
            else:
                nc.vector.scalar_tensor_tensor(x[:], stz, rec2[:], yy,
                                               OP.mult, OP.mult)
            nc.vector.tensor_tensor(u3[:, 0:64, 1], x[:, 0:64], x[:, 64:128],
                                    OP.add)
            if t % RBLK == RBLK - 1:
                ridx = t // RBLK
                mxc = mxh[:, ridx:ridx + 1]
                nc.vector.tensor_reduce(mxc, pnxt[:, 1:130], AX.X, OP.max)
                # rescale so the row max becomes 1.0
                rec2n = spl.tile([128, 1], F32, tag="rec2")
                nc.vector.reciprocal(rec2n[:], mxc)
                return rec2n
            return None

        # init (t = 0): P[s=0] = ybe[:,0]; P~[s=1] = e^-g * y_lab(l=0,t=0)
        nc.vector.tensor_copy(pa[:, 1:2], ybe_sb[:, 0:1])
        nc.vector.tensor_scalar(pa[:, 2:3], ylcs[0][:, 64:65], E1, None, OP.mult)

        pcur, pnxt = pa, pb
        rec2 = None
        for t in range(1, T):
            rec2 = dp_step(t, pcur, pnxt, rec2)
            pcur, pnxt = pnxt, pcur
        if rec2 is not None:
            # the last rescale's scaling never got absorbed; apply it now
            nc.vector.tensor_scalar_mul(pcur[:, 1:130], pcur[:, 1:130], rec2[:])

        # final: export pend = sum(P * endmask) and the rescale history;
        # the exact logs happen on the host.
        scre = per.tile([128, S], F32, tag="scre", name="scre")
        nc.vector.tensor_tensor(scre[:], pcur[:, 1:130], em_sb[:], OP.mult)
        pend = per.tile([128, 1], F32, tag="pend", name="pend")
        nc.vector.tensor_reduce(pend[:], scre[:], AX.X, OP.add)
        nc.sync.dma_start(pend_d, pend[:])
        nc.sync.dma_start(mxh_d, mxh[:])

    nc.compile()
    return nc


def _host_derived(y_true, y_pred, label_length):
    import ml_dtypes

    lab = np.asarray(y_true, dtype=np.int64)          # [B, 64]
    llv = np.asarray(label_length).reshape(-1)
    yp = np.asarray(y_pred, dtype=np.float32)
    # gather label emissions: ylab[b, t, l] = y_pred[b, t, lab[b, l]] + EPS
    ylab = np.take_along_axis(
        yp, np.broadcast_to(lab[:, None, :], (B, T, L)), axis=2
    ) + np.float32(EPS)                                # [B, T, 64] f32
    vm = (np.arange(L)[None, :] < llv[:, None])        # valid odd state s=2l+1
    zm = np.concatenate([np.zeros((B, 1), bool), lab[:, 1:] != lab[:, :-1]],
                        axis=1)
    ck_sk = (np.float32(np.exp(-2.0 * G_TILT)) * (zm & vm)).astype(np.float32)
    ck_lab = vm.astype(np.float32)
    ylc = np.empty((B, T, 128), dtype=ml_dtypes.bfloat16)
    ylc[:, :, 0:64] = ylab * ck_sk[:, None, :]
    ylc[:, :, 64:128] = ylab * ck_lab[:, None, :]
    ybe = np.ascontiguousarray(yp[:, :, C - 1] + np.float32(EPS))
    return ylc, ybe


def kernel(y_true, y_pred, input_length, label_length, _trace=False):
    global _prog, _last_results
    from concourse.bass_utils import run_bass_kernel_spmd

    y_true = np.asarray(y_true)
    label_length = np.asarray(label_length).reshape(-1)

    ylc, ybe = _host_derived(y_true, y_pred, label_length)
    em = np.zeros((B, S), dtype=np.float32)
    bidx = np.arange(B)
    em[bidx, 2 * label_length] = 1.0
    em[bidx, 2 * label_length - 1] = np.float32(np.exp(-G_TILT))

    if _prog is None:
        _prog = _build_program()

    in_maps = []
    for i in range(NCORES):
        sl = slice(i * BL, (i + 1) * BL)
        in_maps.append({
            "ylc": ylc[sl],
            "ybe": ybe[sl],
            "em": em[sl],
        })
    res = run_bass_kernel_spmd(_prog, in_maps, core_ids=list(range(NCORES)),
                               trace=_trace)
    _last_results = res
    pend = np.concatenate([r["pend"] for r in res.results], axis=0).reshape(-1)
    mxh = np.concatenate([r["mxh"] for r in res.results], axis=0)
    logacc = np.log(mxh.astype(np.float64)).sum(axis=1)
    loss = -(np.log(pend.astype(np.float64)) + logacc
             + G_TILT * 2.0 * label_length.astype(np.float64))
    return loss.reshape(B, 1).astype(np.float32)


if __name__ == "__main__":
    rng = np.random.default_rng(0)
    yp = rng.random((B, T, C), dtype=np.float32)
    yp /= yp.sum(-1, keepdims=True)
    yt = rng.integers(0, C - 1, size=(B, L)).astype(np.int32)
    il = np.full((B, 1), T, dtype=np.int32)
    ll = rng.integers(32, L + 1, size=(B, 1)).astype(np.int32)
    print(kernel(yt, yp, il, ll)[:4])


# revision 8
# speedup vs baseline: 3.9476x; 1.0578x over previous
"""CTC loss (keras ctc_batch_cost semantics) on 8 Trainium2 NeuronCores.

Strategy (pure data parallelism, batch sharded 128 samples/core):
  - DP runs in probability space with periodic per-sample rescaling:
        P[t,s] = y_ext[t,s] * (P[t-1,s] + P[t-1,s-1] + allow_skip*P[t-1,s-2])
    Samples ride the 128 SBUF partitions; the S=129 lattice states live in
    the free dimension of [128, S]-shaped DVE ops.
  - The per-(sample,t) label emissions y_pred[b,t,lab(b,l)] are pre-gathered
    on the host into a per-step dictionary ylc[b, t, 0:128]:
        cols 0..63  : e^(-2g) * allow_skip * (y_lab + EPS)   (skip term)
        cols 64..127: valid * (y_lab + EPS)                  (label emission)
    so the device loop is pure DVE work on SBUF-resident tiles - no
    gather matmuls, no transposes, no PSUM.
  - Blank emissions (even lattice states) multiply by a per-partition scalar
    plane ybe[b,t] = y_pred[b,t,C-1]+EPS (tensor_scalar with scalar-AP).
  - Loss = -(log(P[2L] + P[2L-1]) + sum of rescale logs).
"""

import numpy as np

B, T, C, L = 1024, 512, 256, 64
S = 2 * L + 1  # 129
NCORES = 8
BL = B // NCORES  # 128 samples per core
EPS = 1e-7
RBLK = 8  # rescale period (time steps)
# Static per-state exponential tilt P~[s] = P[s]*exp(-G_TILT*s). Flattens the
# lattice's s-profile so all answer-relevant states fit f32 range; folded into
# the sh1 scalar, the host-built dictionary/end-mask, and the final log.
G_TILT = 1.75

_prog = None  # cached compiled Bass program
_last_results = None


def _build_program():
    from contextlib import ExitStack

    import concourse.bacc as bacc
    import concourse.bass as bass
    import concourse.mybir as mybir
    import concourse.tile as tile

    F32 = mybir.dt.float32
    BF16 = mybir.dt.bfloat16
    OP = mybir.AluOpType
    AF = mybir.ActivationFunctionType
    AX = mybir.AxisListType

    TCH = 64             # time-chunk length (per input DMA)
    NCH = T // TCH       # 8 chunks
    E1 = float(np.exp(-G_TILT))

    nc = bacc.Bacc("TRN2", target_bir_lowering=False, debug=False)

    ylc_d = nc.dram_tensor("ylc", [BL, T, 128], BF16, kind="ExternalInput").ap()
    ybe_d = nc.dram_tensor("ybe", [BL, T], F32, kind="ExternalInput").ap()
    em_d = nc.dram_tensor("em", [BL, S], F32, kind="ExternalInput").ap()
    pend_d = nc.dram_tensor("pend", [BL, 1], F32, kind="ExternalOutput").ap()
    mxh_d = nc.dram_tensor("mxh", [BL, T // RBLK], F32, kind="ExternalOutput").ap()

    with tile.TileContext(nc) as tc, ExitStack() as ctx:
        # ---- persistent SBUF state (one pool, unique tags) ----
        per = ctx.enter_context(tc.tile_pool(name="per", bufs=1))
        ybe_sb = per.tile([128, T], F32, tag="ybe", name="ybe_sb")
        em_sb = per.tile([128, S], F32, tag="em", name="em_sb")
        pa = per.tile([128, 264], F32, tag="pa", name="pa")
        pb = per.tile([128, 264], F32, tag="pb", name="pb")
        mxh = per.tile([128, T // RBLK], F32, tag="mxh", name="mxh")
        ylcs = [per.tile([128, TCH * 128], BF16, tag=f"ylc{k}", name=f"ylc{k}")
                for k in range(NCH)]

        nc.sync.dma_start(ybe_sb[:], ybe_d)
        nc.sync.dma_start(em_sb[:], em_d)
        nc.vector.memset(pa[:], 0.0)
        nc.vector.memset(pb[:], 0.0)
        for k in range(NCH):
            nc.sync.dma_start(
                ylcs[k][:],
                ylc_d[:, k * TCH:(k + 1) * TCH, :].rearrange("p t c -> p (t c)"))

        # ---- pools ----
        vpl = ctx.enter_context(tc.tile_pool(name="vpl", bufs=3))
        spl = ctx.enter_context(tc.tile_pool(name="spl", bufs=6))

        AOFF = 134  # A[s] lives at col AOFF+s of the *current* state tensor

        def dp_step(t, pcur, pnxt, rec2):
            k, tl = divmod(t, TCH)
            yy = ylcs[k][:, tl * 128:(tl + 1) * 128]
            # A[s] = P[s] + e^-g*P[s-1], written into pcur's scratch region
            nc.vector.scalar_tensor_tensor(pcur[:, AOFF:AOFF + 129],
                                           pcur[:, 0:129], E1,
                                           pcur[:, 1:130], OP.mult, OP.add)
            u3 = pnxt[:, 1:131].rearrange("p (s two) -> p s two", two=2)
            a_even = pcur[:, AOFF:AOFF + 130].rearrange(
                "p (s two) -> p s two", two=2)[:, :, 0]
            # even states on ScalarE: (A_even * ybe) [* rec2 post-rescale]
            if rec2 is None:
                nc.scalar.activation(u3[:, :, 0], a_even, AF.Copy,
                                     bias=0.0, scale=ybe_sb[:, t:t + 1])
            else:
                berec = spl.tile([128, 1], F32, tag="berec")
                nc.scalar.activation(berec[:], ybe_sb[:, t:t + 1], AF.Copy,
                                     bias=0.0, scale=rec2[:])
                nc.scalar.activation(u3[:, :, 0], a_even, AF.Copy,
                                     bias=0.0, scale=berec[:])
            # one 2D-strided multiply covers skip & label terms:
            #   X[0,l] = P[2l-1] * yy[0..63]   (skip: e^-2g * masked emission)
            #   X[1,l] = A[2l+1] * yy[64..127] (label emission)
            stz = bass.AP(pcur[:].tensor, pcur[:].offset,
                          [pcur[:].ap[0], [AOFF + 1, 2], [2, 64]])
            x = vpl.tile([128, 128], F32, tag="x")
            if rec2 is None:
                nc.vector.tensor_tensor(x[:], stz, yy, OP.mult)
            else:
                nc.vector.scalar_tensor_tensor(x[:], stz, rec2[:], yy,
                                               OP.mult, OP.mult)
            nc.vector.tensor_tensor(u3[:, 0:64, 1], x[:, 0:64], x[:, 64:128],
                                    OP.add)
            if t % RBLK == RBLK - 1:
                ridx = t // RBLK
                mxc = mxh[:, ridx:ridx + 1]
                nc.vector.tensor_reduce(mxc, pnxt[:, 1:130], AX.X, OP.max)
                # rescale so the row max becomes 1.0
                rec2n = spl.tile([128, 1], F32, tag="rec2")
                nc.vector.reciprocal(rec2n[:], mxc)
                return rec2n
            return None

        # init (t = 0): P[s=0] = ybe[:,0]; P~[s=1] = e^-g * y_lab(l=0,t=0)
        nc.vector.tensor_copy(pa[:, 1:2], ybe_sb[:, 0:1])
        nc.vector.tensor_scalar(pa[:, 2:3], ylcs[0][:, 64:65], E1, None, OP.mult)

        pcur, pnxt = pa, pb
        rec2 = None
        for t in range(1, T):
            rec2 = dp_step(t, pcur, pnxt, rec2)
            pcur, pnxt = pnxt, pcur
        if rec2 is not None:
            # the last rescale's scaling never got absorbed; apply it now
            nc.vector.tensor_scalar_mul(pcur[:, 1:130], pcur[:, 1:130], rec2[:])

        # final: export pend = sum(P * endmask) and the rescale history;
        # the exact logs happen on the host.
        scre = per.tile([128, S], F32, tag="scre", name="scre")
        nc.vector.tensor_tensor(scre[:], pcur[:, 1:130], em_sb[:], OP.mult)
        pend = per.tile([128, 1], F32, tag="pend", name="pend")
        nc.vector.tensor_reduce(pend[:], scre[:], AX.X, OP.add)
        nc.sync.dma_start(pend_d, pend[:])
        nc.sync.dma_start(mxh_d, mxh[:])

    nc.compile()
    return nc


def _host_derived(y_true, y_pred, label_length):
    import ml_dtypes

    lab = np.asarray(y_true, dtype=np.int64)          # [B, 64]
    llv = np.asarray(label_length).reshape(-1)
    yp = np.asarray(y_pred, dtype=np.float32)
    # gather label emissions: ylab[b, t, l] = y_pred[b, t, lab[b, l]] + EPS
    ylab = np.take_along_axis(
        yp, np.broadcast_to(lab[:, None, :], (B, T, L)), axis=2
    ) + np.float32(EPS)                                # [B, T, 64] f32
    vm = (np.arange(L)[None, :] < llv[:, None])        # valid odd state s=2l+1
    zm = np.concatenate([np.zeros((B, 1), bool), lab[:, 1:] != lab[:, :-1]],
                        axis=1)
    ck_sk = (np.float32(np.exp(-2.0 * G_TILT)) * (zm & vm)).astype(np.float32)
    ck_lab = vm.astype(np.float32)
    ylc = np.empty((B, T, 128), dtype=ml_dtypes.bfloat16)
    ylc[:, :, 0:64] = ylab * ck_sk[:, None, :]
    ylc[:, :, 64:128] = ylab * ck_lab[:, None, :]
    ybe = np.ascontiguousarray(yp[:, :, C - 1] + np.float32(EPS))
    return ylc, ybe


def kernel(y_true, y_pred, input_length, label_length, _trace=False):
    global _prog, _last_results
    from concourse.bass_utils import run_bass_kernel_spmd

    y_true = np.asarray(y_true)
    label_length = np.asarray(label_length).reshape(-1)

    ylc, ybe = _host_derived(y_true, y_pred, label_length)
    em = np.zeros((B, S), dtype=np.float32)
    bidx = np.arange(B)
    em[bidx, 2 * label_length] = 1.0
    em[bidx, 2 * label_length - 1] = np.float32(np.exp(-G_TILT))

    if _prog is None:
        _prog = _build_program()

    in_maps = []
    for i in range(NCORES):
        sl = slice(i * BL, (i + 1) * BL)
        in_maps.append({
            "ylc": ylc[sl],
            "ybe": ybe[sl],
            "em": em[sl],
        })
    res = run_bass_kernel_spmd(_prog, in_maps, core_ids=list(range(NCORES)),
                               trace=_trace)
    _last_results = res
    pend = np.concatenate([r["pend"] for r in res.results], axis=0).reshape(-1)
    mxh = np.concatenate([r["mxh"] for r in res.results], axis=0)
    logacc = np.log(mxh.astype(np.float64)).sum(axis=1)
    loss = -(np.log(pend.astype(np.float64)) + logacc
             + G_TILT * 2.0 * label_length.astype(np.float64))
    return loss.reshape(B, 1).astype(np.float32)


if __name__ == "__main__":
    rng = np.random.default_rng(0)
    yp = rng.random((B, T, C), dtype=np.float32)
    yp /= yp.sum(-1, keepdims=True)
    yt = rng.integers(0, C - 1, size=(B, L)).astype(np.int32)
    il = np.full((B, 1), T, dtype=np.int32)
    ll = rng.integers(32, L + 1, size=(B, 1)).astype(np.int32)
    print(kernel(yt, yp, il, ll)[:4])
